# revision 7
# baseline (speedup 1.0000x reference)
"""Chunked cross-attention Trainium2 kernel.

Reference computation (fp32):
  q_in  = query.reshape(B, NC, 64, D)           B=4, NC=32, D=1024
  kv_in = key.reshape(B, NC, 256, D)
  q = (q_in @ Wq + bq)  split into H=16 heads of d_k=64
  k = (kv_in @ Wk + bk), v = (kv_in @ Wv + bv)
  o = softmax(q k^T / 8) v   per (b, chunk, head)
  out = merge_heads(o) @ Wo + bo

Sharding: data-parallel over the 128 independent (b, chunk) pairs,
16 pairs per core, SPMD on 8 cores, no collectives.

Math notes used below:
  * bk is exactly a no-op: scores[q, j] = q_vec . (k_unb[j] + bk) and the
    q_vec . bk term is constant in j, so softmax cancels it.
  * bv and bo are exact post-additive constants: softmax rows sum to 1,
    so o = attn @ (v_unb + bv) = attn @ v_unb + bv, and the final output
    gets + (bv @ Wo + bo), added on the host in fp32.
  * bq folds into the Q-projection PSUM->SBUF copy (per-partition add).

On-chip layouts (per core):
  qT   [128, 8, 1024]  qT[p, ki, t] = q_in^T[ki*128+p, t]   (feature-major)
  kvT  [128, 8, 4096]  same for kv tokens
  w*   [128, 8, 1024]  W[ki*128+p, o]  (natural [in, out])
  Q/K projections emit feature-major [o, t] (o on partitions), which is
  exactly the lhsT/rhs layout the scores matmul needs per head.
  V projection emits token-major [t, o] (tokens on partitions) which is
  the lhsT layout of the attn@V matmul.
  probs are transposed with PE-transpose; O-projection emits out^T
  [D, t] which the host transposes back.
"""

import os

import numpy as np
import ml_dtypes

import concourse.bass as bass
import concourse.mybir as mybir
import concourse.tile as tile
from concourse import bacc
from concourse.bass_utils import run_bass_kernel_spmd
from concourse.masks import make_identity

# ---------------------------------------------------------------- constants
N_CORES = 8
B, S, D = 4, 2048, 1024
NCHUNK, CHUNK, L = 32, 64, 256           # chunks per batch, q len, kv len
H, DK = 16, 64                           # heads, head dim
PAIRS = B * NCHUNK                       # 128 independent (b, chunk) pairs
PPC = PAIRS // N_CORES                   # 16 pairs per core
G = 4                                    # pairs per on-chip group
NG = PPC // G                            # 4 groups
TQ = PPC * CHUNK                         # 1024 q tokens per core
TKV = PPC * L                            # 4096 kv tokens per core
GQ = G * CHUNK                           # 256 q tokens per group
GKV = G * L                              # 1024 kv tokens per group
KI = D // 128                            # 8 contraction tiles
OT = (H * DK) // 128                     # 8 output-feature tiles (2 heads each)

MM_DT = mybir.dt.bfloat16                # matmul operand dtype
NP_DT = ml_dtypes.bfloat16
F32 = mybir.dt.float32

_BUILD_CACHE: dict = {}
LAST_RESULTS = None                      # BassKernelResults of last run (debug)


def _install_ntff_hook_shim():
    """Provide antenv.axon_hooks if the image lacks it (trace-only path).

    Mirrors trn_agent_boot's ctypes NTFF hook against the injected
    libaxon_pjrt.so so run_bass_kernel_spmd(trace=True) can capture
    hardware profiles under axon.
    """
    import sys
    import types
    import ctypes
    import contextlib

    try:
        from antenv.axon_hooks import get_axon_ntff_profile_hook  # noqa: F401
        return
    except ImportError:
        pass

    so_path = "/opt/axon/libaxon_pjrt.so"
    if not os.path.exists(so_path):
        return
    lib = ctypes.CDLL(so_path)
    if not hasattr(lib, "axon_start_nrt_profile"):
        return
    lib.axon_start_nrt_profile.argtypes = [
        ctypes.POINTER(ctypes.c_int64),
        ctypes.c_size_t,
    ]
    lib.axon_start_nrt_profile.restype = ctypes.c_int64
    lib.axon_stop_nrt_profile.argtypes = [ctypes.c_char_p]
    lib.axon_stop_nrt_profile.restype = ctypes.c_int64

    @contextlib.contextmanager
    def _hook(output_dir, device_ids):
        import jax

        jax.devices()
        if device_ids:
            ids = (ctypes.c_int64 * len(device_ids))(*device_ids)
            rc = lib.axon_start_nrt_profile(ids, len(device_ids))
        else:
            rc = lib.axon_start_nrt_profile(None, 0)
        if rc != 0:
            raise RuntimeError(f"axon_start_nrt_profile rc={rc}")
        try:
            yield
        finally:
            n = lib.axon_stop_nrt_profile(str(output_dir).encode())
            print(f"ntff profile: {n} file(s) written to {output_dir}",
                  file=sys.stderr)

    mod = types.ModuleType("antenv.axon_hooks")
    mod.get_axon_ntff_profile_hook = lambda: _hook
    mod.set_axon_ntff_profile_hook = lambda h: None
    import antenv

    antenv.axon_hooks = mod
    sys.modules["antenv.axon_hooks"] = mod


def _emit(nc: bass.Bass, tc, qT_d, kvT_d, wq_d, wk_d, wv_d, wo_d, outT_d):
    import contextlib

    ctx = contextlib.ExitStack()
    with ctx:
        const = ctx.enter_context(tc.tile_pool(name="const", bufs=1))
        w_q = const.tile([128, KI, 1024], MM_DT)
        w_k = const.tile([128, KI, 1024], MM_DT)
        w_v = const.tile([128, KI, 1024], MM_DT)
        w_o = const.tile([128, KI, 1024], MM_DT)
        ident = const.tile([128, 128], MM_DT)
        nc.sync.dma_start(w_q[:], wq_d[:])
        nc.sync.dma_start(w_k[:], wk_d[:])
        nc.sync.dma_start(w_v[:], wv_d[:])
        nc.sync.dma_start(w_o[:], wo_d[:])
        make_identity(nc, ident[:])

        qio = ctx.enter_context(tc.tile_pool(name="qio", bufs=1))
        qT = qio.tile([128, KI, TQ], MM_DT)
        nc.sync.dma_start(qT[:], qT_d[:])
        qprojT = qio.tile([128, OT, TQ], MM_DT)

        kvin = ctx.enter_context(tc.tile_pool(name="kvin", bufs=2))
        vpp = ctx.enter_context(tc.tile_pool(name="vpp", bufs=2))
        kpp = ctx.enter_context(tc.tile_pool(name="kpp", bufs=3))
        att = ctx.enter_context(tc.tile_pool(name="att", bufs=4))
        otp = ctx.enter_context(tc.tile_pool(name="otp", bufs=2))
        outp = ctx.enter_context(tc.tile_pool(name="outp", bufs=2))
        psA = ctx.enter_context(tc.tile_pool(name="psA", bufs=4, space="PSUM"))
        psB = ctx.enter_context(tc.tile_pool(name="psB", bufs=4, space="PSUM"))

        # ---------------- Q projection, all 16 pairs at once -------------
        # qprojT[o, t] : o on partitions (2 heads per ot tile), t free
        for ot in range(OT):
            for c in range(TQ // 512):
                ps_q = psA.tile([128, 512], F32, tag="psA")
                for ki in range(KI):
                    nc.tensor.matmul(
                        ps_q[:],
                        w_q[:, ki, ot * 128:(ot + 1) * 128],
                        qT[:, ki, c * 512:(c + 1) * 512],
                        start=(ki == 0),
                        stop=(ki == KI - 1),
                    )
                nc.vector.tensor_copy(
                    qprojT[:, ot, c * 512:(c + 1) * 512], ps_q[:])

        # ---------------- per-group KV pipeline ---------------------------
        for g in range(NG):
            kvT_g = kvin.tile([128, KI, GKV], MM_DT, tag="kvT_g")
            nc.sync.dma_start(kvT_g[:], kvT_d[:, :, g * GKV:(g + 1) * GKV])

            # V projection, token-major: vproj[t, o], tokens on partitions
            vproj_g = vpp.tile([128, 2 * G, 1024], MM_DT, tag="vproj_g")
            for ts in range(2 * G):
                for oc in range(2):
                    ps_v = psA.tile([128, 512], F32, tag="psA")
                    for ki in range(KI):
                        nc.tensor.matmul(
                            ps_v[:],
                            kvT_g[:, ki, ts * 128:(ts + 1) * 128],
                            w_v[:, ki, oc * 512:(oc + 1) * 512],
                            start=(ki == 0),
                            stop=(ki == KI - 1),
                        )
                    nc.vector.tensor_copy(
                        vproj_g[:, ts, oc * 512:(oc + 1) * 512], ps_v[:])

            oT_g = otp.tile([128, OT, GQ], MM_DT, tag="oT_g")

            for ot in range(OT):
                # K projection for this feature tile (2 heads), feature-major
                kprojT = kpp.tile([128, GKV], MM_DT, tag="kprojT")
                for c in range(GKV // 512):
                    ps_k = psA.tile([128, 512], F32, tag="psA")
                    for ki in range(KI):
                        nc.tensor.matmul(
                            ps_k[:],
                            w_k[:, ki, ot * 128:(ot + 1) * 128],
                            kvT_g[:, ki, c * 512:(c + 1) * 512],
                            start=(ki == 0),
                            stop=(ki == KI - 1),
                        )
                    nc.vector.tensor_copy(
                        kprojT[:, c * 512:(c + 1) * 512], ps_k[:])

                for p in range(G):
                    pg = g * G + p       # global pair index on this core
                    # scores, 2 heads packed on the PE (row+col tiles)
                    ps_s = psB.tile([128, 256], F32, tag="psB")
                    for h in range(2):
                        sl = slice(h * 64, (h + 1) * 64)
                        nc.tensor.matmul(
                            ps_s[sl, :],
                            qprojT[sl, ot, pg * 64:(pg + 1) * 64],
                            kprojT[sl, p * 256:(p + 1) * 256],
                            start=True, stop=True,
                            tile_position=(h * 64, h * 64),
                        )
                    # softmax along free axis; scores ~ N(0,1), no max-sub
                    probs = att.tile([128, 256], MM_DT, tag="probs")
                    sums = att.tile([128, 1], F32, tag="sums")
                    nc.scalar.activation(
                        probs[:], ps_s[:],
                        mybir.ActivationFunctionType.Exp,
                        bias=0.0, scale=1.0 / np.sqrt(DK).item(),
                        accum_out=sums[:],
                    )
                    rs = att.tile([128, 1], F32, tag="rs")
                    nc.vector.reciprocal(rs[:], sums[:])
                    nc.vector.tensor_scalar_mul(probs[:], probs[:], rs[:])
                    # transpose probs -> probsT [kv, q2]
                    ps_t = psB.tile([128, 256], MM_DT, tag="psB")
                    nc.tensor.transpose(ps_t[:, 0:128], probs[:, 0:128], ident[:])
                    nc.tensor.transpose(ps_t[:, 128:256], probs[:, 128:256], ident[:])
                    probsT = att.tile([128, 256], MM_DT, tag="probsT")
                    nc.vector.tensor_copy(probsT[:], ps_t[:])
                    # attn @ V -> o^T [dv2, q], 2 heads on distinct col tiles
                    ps_o = psB.tile([128, 64], F32, tag="psB")
                    for h in range(2):
                        for kt in range(2):
                            nc.tensor.matmul(
                                ps_o[h * 64:(h + 1) * 64, :],
                                vproj_g[:, p * 2 + kt,
                                        ot * 128 + h * 64: ot * 128 + (h + 1) * 64],
                                probsT[:, kt * 128 + h * 64: kt * 128 + (h + 1) * 64],
                                start=(kt == 0), stop=(kt == 1),
                                tile_position=(0, h * 64),
                            )
                    nc.vector.tensor_copy(oT_g[:, ot, p * 64:(p + 1) * 64], ps_o[:])

            # O projection: out^T [D, t], D on partitions
            outT_g = outp.tile([128, OT, GQ], F32, tag="outT_g")
            for dt in range(OT):
                ps_f = psB.tile([128, 256], F32, tag="psB")
                for ki in range(KI):
                    nc.tensor.matmul(
                        ps_f[:],
                        w_o[:, ki, dt * 128:(dt + 1) * 128],
                        oT_g[:, ki, :],
                        start=(ki == 0),
                        stop=(ki == KI - 1),
                    )
                nc.vector.tensor_copy(outT_g[:, dt, :], ps_f[:])
            nc.sync.dma_start(outT_d[:, :, g * GQ:(g + 1) * GQ], outT_g[:])


def _build():
    key = "k"
    if key in _BUILD_CACHE:
        return _BUILD_CACHE[key]
    nc = bacc.Bacc("TRN2", target_bir_lowering=False, debug=False)
    qT_d = nc.dram_tensor("qT", [128, KI, TQ], MM_DT, kind="ExternalInput").ap()
    kvT_d = nc.dram_tensor("kvT", [128, KI, TKV], MM_DT, kind="ExternalInput").ap()
    wq_d = nc.dram_tensor("wq", [128, KI, 1024], MM_DT, kind="ExternalInput").ap()
    wk_d = nc.dram_tensor("wk", [128, KI, 1024], MM_DT, kind="ExternalInput").ap()
    wv_d = nc.dram_tensor("wv", [128, KI, 1024], MM_DT, kind="ExternalInput").ap()
    wo_d = nc.dram_tensor("wo", [128, KI, 1024], MM_DT, kind="ExternalInput").ap()
    outT_d = nc.dram_tensor("outT", [128, OT, TQ], F32, kind="ExternalOutput").ap()
    with tile.TileContext(nc) as tc:
        _emit(nc, tc, qT_d, kvT_d, wq_d, wk_d, wv_d, wo_d, outT_d)
    nc.compile()
    _BUILD_CACHE[key] = nc
    return nc


def _feat_major(x2d: np.ndarray) -> np.ndarray:
    """[tokens, D] fp32 -> [128, KI, tokens] in MM dtype (d = ki*128 + p)."""
    t, d = x2d.shape
    return np.ascontiguousarray(
        x2d.T.reshape(KI, 128, t).transpose(1, 0, 2)).astype(NP_DT)


def kernel(query, key, Wq, bq, Wk, bk, Wv, bv, Wo, bo):
    global LAST_RESULTS
    query = np.asarray(query, dtype=np.float32)
    key = np.asarray(key, dtype=np.float32)
    Wq = np.asarray(Wq, dtype=np.float32)
    Wk = np.asarray(Wk, dtype=np.float32)
    Wv = np.asarray(Wv, dtype=np.float32)
    Wo = np.asarray(Wo, dtype=np.float32)
    bq = np.asarray(bq, dtype=np.float32)
    bv = np.asarray(bv, dtype=np.float32)
    bo = np.asarray(bo, dtype=np.float32)

    q_pairs = query.reshape(PAIRS, CHUNK, D)
    if np.any(bq):
        # fold bq into the q tokens: (q + delta) @ Wq == q @ Wq + bq
        delta = np.linalg.solve(Wq.astype(np.float64).T,
                                bq.astype(np.float64)).astype(np.float32)
        q_pairs = q_pairs + delta[None, None, :]
    kv_pairs = key.reshape(PAIRS, L, D)

    w_args = {
        "wq": np.ascontiguousarray(
            Wq.reshape(KI, 128, H * DK).transpose(1, 0, 2)).astype(NP_DT),
        "wk": np.ascontiguousarray(
            Wk.reshape(KI, 128, H * DK).transpose(1, 0, 2)).astype(NP_DT),
        "wv": np.ascontiguousarray(
            Wv.reshape(KI, 128, H * DK).transpose(1, 0, 2)).astype(NP_DT),
        "wo": np.ascontiguousarray(
            Wo.reshape(KI, 128, D).transpose(1, 0, 2)).astype(NP_DT),
    }

    in_maps = []
    for c in range(N_CORES):
        sl = slice(c * PPC, (c + 1) * PPC)
        m = dict(w_args)
        m["qT"] = _feat_major(q_pairs[sl].reshape(TQ, D))
        m["kvT"] = _feat_major(kv_pairs[sl].reshape(TKV, D))
        in_maps.append(m)

    nc = _build()
    trace = os.environ.get("CCA_TRACE", "") == "1"
    if trace:
        try:
            _install_ntff_hook_shim()
            tmpdir = os.environ.get("CCA_TRACE_DIR") or None
            if tmpdir:
                os.makedirs(tmpdir, exist_ok=True)
            res = run_bass_kernel_spmd(
                nc, in_maps, list(range(N_CORES)), trace=True, tmpdir=tmpdir)
        except Exception as e:
            print(f"trace path failed ({e!r}); rerunning without trace")
            res = run_bass_kernel_spmd(nc, in_maps, list(range(N_CORES)))
    else:
        res = run_bass_kernel_spmd(nc, in_maps, list(range(N_CORES)))
    LAST_RESULTS = res

    # gather: outT [128, OT, TQ] -> [t, D] per core
    out = np.empty((PAIRS, CHUNK, D), dtype=np.float32)
    for c in range(N_CORES):
        oT = res.results[c]["outT"]           # [128, OT, TQ]
        full = oT.transpose(1, 0, 2).reshape(D, TQ)  # [D, t]
        out[c * PPC:(c + 1) * PPC] = full.T.reshape(PPC, CHUNK, D)

    # exact bias correction: bk is a softmax no-op; bv/bo are additive.
    out += (bv.astype(np.float64) @ Wo.astype(np.float64) + bo).astype(
        np.float32)[None, None, :]
    return out.reshape(B, S, D)


# revision 8
# speedup vs baseline: 1.2395x; 1.2395x over previous
"""Chunked cross-attention Trainium2 kernel.

Reference computation (fp32):
  q_in  = query.reshape(B, NC, 64, D)           B=4, NC=32, D=1024
  kv_in = key.reshape(B, NC, 256, D)
  q = (q_in @ Wq + bq)  split into H=16 heads of d_k=64
  k = (kv_in @ Wk + bk), v = (kv_in @ Wv + bv)
  o = softmax(q k^T / 8) v   per (b, chunk, head)
  out = merge_heads(o) @ Wo + bo

Sharding: data-parallel over the 128 independent (b, chunk) pairs,
16 pairs per core, SPMD on 8 cores, no collectives.

Math notes used below:
  * bk is exactly a no-op: scores[q, j] = q_vec . (k_unb[j] + bk) and the
    q_vec . bk term is constant in j, so softmax cancels it.
  * bv and bo are exact post-additive constants: softmax rows sum to 1,
    so o = attn @ (v_unb + bv) = attn @ v_unb + bv, and the final output
    gets + (bv @ Wo + bo), added on the host in fp32.
  * bq folds into the Q-projection PSUM->SBUF copy (per-partition add).

On-chip layouts (per core):
  qT   [128, 8, 1024]  qT[p, ki, t] = q_in^T[ki*128+p, t]   (feature-major)
  kvT  [128, 8, 4096]  same for kv tokens
  w*   [128, 8, 1024]  W[ki*128+p, o]  (natural [in, out])
  Q/K projections emit feature-major [o, t] (o on partitions), which is
  exactly the lhsT/rhs layout the scores matmul needs per head.
  V projection emits token-major [t, o] (tokens on partitions) which is
  the lhsT layout of the attn@V matmul.
  probs are transposed with PE-transpose; O-projection emits out^T
  [D, t] which the host transposes back.
"""

import os

import numpy as np
import ml_dtypes

import concourse.bass as bass
import concourse.mybir as mybir
import concourse.tile as tile
from concourse import bacc
from concourse.bass_utils import run_bass_kernel_spmd
from concourse.masks import make_identity

# ---------------------------------------------------------------- constants
N_CORES = 8
B, S, D = 4, 2048, 1024
NCHUNK, CHUNK, L = 32, 64, 256           # chunks per batch, q len, kv len
H, DK = 16, 64                           # heads, head dim
PAIRS = B * NCHUNK                       # 128 independent (b, chunk) pairs
PPC = PAIRS // N_CORES                   # 16 pairs per core
G = 4                                    # pairs per on-chip group
NG = PPC // G                            # 4 groups
TQ = PPC * CHUNK                         # 1024 q tokens per core
TKV = PPC * L                            # 4096 kv tokens per core
GQ = G * CHUNK                           # 256 q tokens per group
GKV = G * L                              # 1024 kv tokens per group
KI = D // 128                            # 8 contraction tiles
OT = (H * DK) // 128                     # 8 output-feature tiles (2 heads each)

MM_DT = mybir.dt.bfloat16                # matmul operand dtype
NP_DT = ml_dtypes.bfloat16
F32 = mybir.dt.float32

_BUILD_CACHE: dict = {}
LAST_RESULTS = None                      # BassKernelResults of last run (debug)


def _install_ntff_hook_shim():
    """Provide antenv.axon_hooks if the image lacks it (trace-only path).

    Mirrors trn_agent_boot's ctypes NTFF hook against the injected
    libaxon_pjrt.so so run_bass_kernel_spmd(trace=True) can capture
    hardware profiles under axon.
    """
    import sys
    import types
    import ctypes
    import contextlib

    try:
        from antenv.axon_hooks import get_axon_ntff_profile_hook  # noqa: F401
        return
    except ImportError:
        pass

    so_path = "/opt/axon/libaxon_pjrt.so"
    if not os.path.exists(so_path):
        return
    lib = ctypes.CDLL(so_path)
    if not hasattr(lib, "axon_start_nrt_profile"):
        return
    lib.axon_start_nrt_profile.argtypes = [
        ctypes.POINTER(ctypes.c_int64),
        ctypes.c_size_t,
    ]
    lib.axon_start_nrt_profile.restype = ctypes.c_int64
    lib.axon_stop_nrt_profile.argtypes = [ctypes.c_char_p]
    lib.axon_stop_nrt_profile.restype = ctypes.c_int64

    @contextlib.contextmanager
    def _hook(output_dir, device_ids):
        import jax

        jax.devices()
        if device_ids:
            ids = (ctypes.c_int64 * len(device_ids))(*device_ids)
            rc = lib.axon_start_nrt_profile(ids, len(device_ids))
        else:
            rc = lib.axon_start_nrt_profile(None, 0)
        if rc != 0:
            raise RuntimeError(f"axon_start_nrt_profile rc={rc}")
        try:
            yield
        finally:
            n = lib.axon_stop_nrt_profile(str(output_dir).encode())
            print(f"ntff profile: {n} file(s) written to {output_dir}",
                  file=sys.stderr)

    mod = types.ModuleType("antenv.axon_hooks")
    mod.get_axon_ntff_profile_hook = lambda: _hook
    mod.set_axon_ntff_profile_hook = lambda h: None
    import antenv

    antenv.axon_hooks = mod
    sys.modules["antenv.axon_hooks"] = mod


def _emit(nc: bass.Bass, tc, qT_d, kvT_d, wq_d, wk_d, wv_d, wo_d, outT_d):
    import contextlib

    ctx = contextlib.ExitStack()
    with ctx:
        const = ctx.enter_context(tc.tile_pool(name="const", bufs=1))
        w_v = const.tile([128, KI, 1024], MM_DT)
        w_k = const.tile([128, KI, 1024], MM_DT)
        w_q = const.tile([128, KI, 1024], MM_DT)
        w_o = const.tile([128, KI, 1024], MM_DT)
        ident = const.tile([128, 128], MM_DT)
        # w_v first: V-proj of group 0 is the first PE work and only needs
        # w_v + the first kv slab; the rest loads behind it.
        nc.sync.dma_start(w_v[:], wv_d[:])
        nc.sync.dma_start(w_k[:], wk_d[:])
        nc.sync.dma_start(w_q[:], wq_d[:])
        nc.sync.dma_start(w_o[:], wo_d[:])
        make_identity(nc, ident[:])

        qio = ctx.enter_context(tc.tile_pool(name="qio", bufs=1))
        qT = qio.tile([128, KI, TQ], MM_DT)
        nc.sync.dma_start(qT[:], qT_d[:])
        qprojT = qio.tile([128, OT, TQ], MM_DT)

        kvin = ctx.enter_context(tc.tile_pool(name="kvin", bufs=2))
        vpp = ctx.enter_context(tc.tile_pool(name="vpp", bufs=2))
        kpp = ctx.enter_context(tc.tile_pool(name="kpp", bufs=3))
        att = ctx.enter_context(tc.tile_pool(name="att", bufs=6))
        otp = ctx.enter_context(tc.tile_pool(name="otp", bufs=2))
        outp = ctx.enter_context(tc.tile_pool(name="outp", bufs=2))
        # PSUM: 8 banks total = psA(2) + ps_s(3) + att2(3)
        psA = ctx.enter_context(tc.tile_pool(name="psA", bufs=2, space="PSUM"))
        psS = ctx.enter_context(tc.tile_pool(name="psS", bufs=3, space="PSUM"))
        psT = ctx.enter_context(tc.tile_pool(name="psT", bufs=3, space="PSUM"))

        def q_projection():
            # qprojT[o, t] : o on partitions (2 heads per ot tile), t free
            for ot in range(OT):
                for c in range(TQ // 512):
                    ps_q = psA.tile([128, 512], F32, tag="psA")
                    for ki in range(KI):
                        nc.tensor.matmul(
                            ps_q[:],
                            w_q[:, ki, ot * 128:(ot + 1) * 128],
                            qT[:, ki, c * 512:(c + 1) * 512],
                            start=(ki == 0),
                            stop=(ki == KI - 1),
                        )
                    nc.vector.tensor_copy(
                        qprojT[:, ot, c * 512:(c + 1) * 512], ps_q[:])

        # ---------------- per-group KV pipeline ---------------------------
        for g in range(NG):
            kvT_g = kvin.tile([128, KI, GKV], MM_DT, tag="kvT_g")
            nc.sync.dma_start(kvT_g[:], kvT_d[:, :, g * GKV:(g + 1) * GKV])

            # V projection, token-major: vproj[t, o], tokens on partitions
            vproj_g = vpp.tile([128, 2 * G, 1024], MM_DT, tag="vproj_g")
            for ts in range(2 * G):
                for oc in range(2):
                    ps_v = psA.tile([128, 512], F32, tag="psA")
                    for ki in range(KI):
                        nc.tensor.matmul(
                            ps_v[:],
                            kvT_g[:, ki, ts * 128:(ts + 1) * 128],
                            w_v[:, ki, oc * 512:(oc + 1) * 512],
                            start=(ki == 0),
                            stop=(ki == KI - 1),
                        )
                    nc.vector.tensor_copy(
                        vproj_g[:, ts, oc * 512:(oc + 1) * 512], ps_v[:])

            if g == 0:
                # behind V-proj g0 so its weight/qT DMAs are covered
                q_projection()

            oT_g = otp.tile([128, OT, GQ], MM_DT, tag="oT_g")

            for ot in range(OT):
                # K projection for this feature tile (2 heads), feature-major
                kprojT = kpp.tile([128, GKV], MM_DT, tag="kprojT")
                for c in range(GKV // 512):
                    ps_k = psA.tile([128, 512], F32, tag="psA")
                    for ki in range(KI):
                        nc.tensor.matmul(
                            ps_k[:],
                            w_k[:, ki, ot * 128:(ot + 1) * 128],
                            kvT_g[:, ki, c * 512:(c + 1) * 512],
                            start=(ki == 0),
                            stop=(ki == KI - 1),
                        )
                    nc.vector.tensor_copy(
                        kprojT[:, c * 512:(c + 1) * 512], ps_k[:])

                # ---- phase 1: scores + softmax for all G pairs -----------
                probs_l = []
                for p in range(G):
                    pg = g * G + p       # global pair index on this core
                    ps_s = psS.tile([128, 256], F32, tag="psS")
                    for h in range(2):
                        sl = slice(h * 64, (h + 1) * 64)
                        nc.tensor.matmul(
                            ps_s[sl, :],
                            qprojT[sl, ot, pg * 64:(pg + 1) * 64],
                            kprojT[sl, p * 256:(p + 1) * 256],
                            start=True, stop=True,
                            tile_position=(h * 64, h * 64),
                        )
                    # softmax along free axis; scores ~ N(0,1), no max-sub
                    probs = att.tile([128, 256], MM_DT, tag="probs")
                    sums = att.tile([128, 1], F32, tag="sums")
                    nc.scalar.activation(
                        probs[:], ps_s[:],
                        mybir.ActivationFunctionType.Exp,
                        bias=0.0, scale=1.0 / np.sqrt(DK).item(),
                        accum_out=sums[:],
                    )
                    rs = att.tile([128, 1], F32, tag="rs")
                    nc.vector.reciprocal(rs[:], sums[:])
                    nc.vector.tensor_scalar_mul(probs[:], probs[:], rs[:])
                    probs_l.append(probs)

                # ---- phase 2: transpose(p) | AV(p-1), lag-1 pipeline ----
                probsT_l = [None] * G
                for p in range(G + 1):
                    if p < G:
                        ps_t = psT.tile([128, 256], MM_DT, tag="psT")
                        nc.tensor.transpose(
                            ps_t[:, 0:128], probs_l[p][:, 0:128], ident[:])
                        nc.tensor.transpose(
                            ps_t[:, 128:256], probs_l[p][:, 128:256], ident[:])
                        probsT = att.tile([128, 256], MM_DT, tag="probsT")
                        nc.vector.tensor_copy(probsT[:], ps_t[:])
                        probsT_l[p] = probsT
                    if p >= 1:
                        pa = p - 1
                        # attn @ V -> o^T [dv2, q], 2 heads on col tiles
                        ps_o = psT.tile([128, 64], F32, tag="psT")
                        for h in range(2):
                            for kt in range(2):
                                nc.tensor.matmul(
                                    ps_o[h * 64:(h + 1) * 64, :],
                                    vproj_g[:, pa * 2 + kt,
                                            ot * 128 + h * 64:
                                            ot * 128 + (h + 1) * 64],
                                    probsT_l[pa][:, kt * 128 + h * 64:
                                                  kt * 128 + (h + 1) * 64],
                                    start=(kt == 0), stop=(kt == 1),
                                    tile_position=(0, h * 64),
                                )
                        nc.vector.tensor_copy(
                            oT_g[:, ot, pa * 64:(pa + 1) * 64], ps_o[:])

            # O projection: out^T [D, t], D on partitions
            outT_g = outp.tile([128, OT, GQ], F32, tag="outT_g")
            for dt in range(OT):
                ps_f = psA.tile([128, 256], F32, tag="psA")
                for ki in range(KI):
                    nc.tensor.matmul(
                        ps_f[:],
                        w_o[:, ki, dt * 128:(dt + 1) * 128],
                        oT_g[:, ki, :],
                        start=(ki == 0),
                        stop=(ki == KI - 1),
                    )
                nc.vector.tensor_copy(outT_g[:, dt, :], ps_f[:])
            nc.sync.dma_start(outT_d[:, :, g * GQ:(g + 1) * GQ], outT_g[:])


def _build():
    key = "k"
    if key in _BUILD_CACHE:
        return _BUILD_CACHE[key]
    nc = bacc.Bacc("TRN2", target_bir_lowering=False, debug=False)
    qT_d = nc.dram_tensor("qT", [128, KI, TQ], MM_DT, kind="ExternalInput").ap()
    kvT_d = nc.dram_tensor("kvT", [128, KI, TKV], MM_DT, kind="ExternalInput").ap()
    wq_d = nc.dram_tensor("wq", [128, KI, 1024], MM_DT, kind="ExternalInput").ap()
    wk_d = nc.dram_tensor("wk", [128, KI, 1024], MM_DT, kind="ExternalInput").ap()
    wv_d = nc.dram_tensor("wv", [128, KI, 1024], MM_DT, kind="ExternalInput").ap()
    wo_d = nc.dram_tensor("wo", [128, KI, 1024], MM_DT, kind="ExternalInput").ap()
    outT_d = nc.dram_tensor("outT", [128, OT, TQ], F32, kind="ExternalOutput").ap()
    with tile.TileContext(nc) as tc:
        _emit(nc, tc, qT_d, kvT_d, wq_d, wk_d, wv_d, wo_d, outT_d)
    nc.compile()
    _BUILD_CACHE[key] = nc
    return nc


def _feat_major(x2d: np.ndarray) -> np.ndarray:
    """[tokens, D] fp32 -> [128, KI, tokens] in MM dtype (d = ki*128 + p)."""
    t, d = x2d.shape
    return np.ascontiguousarray(
        x2d.T.reshape(KI, 128, t).transpose(1, 0, 2)).astype(NP_DT)


def kernel(query, key, Wq, bq, Wk, bk, Wv, bv, Wo, bo):
    global LAST_RESULTS
    query = np.asarray(query, dtype=np.float32)
    key = np.asarray(key, dtype=np.float32)
    Wq = np.asarray(Wq, dtype=np.float32)
    Wk = np.asarray(Wk, dtype=np.float32)
    Wv = np.asarray(Wv, dtype=np.float32)
    Wo = np.asarray(Wo, dtype=np.float32)
    bq = np.asarray(bq, dtype=np.float32)
    bv = np.asarray(bv, dtype=np.float32)
    bo = np.asarray(bo, dtype=np.float32)

    q_pairs = query.reshape(PAIRS, CHUNK, D)
    if np.any(bq):
        # fold bq into the q tokens: (q + delta) @ Wq == q @ Wq + bq
        delta = np.linalg.solve(Wq.astype(np.float64).T,
                                bq.astype(np.float64)).astype(np.float32)
        q_pairs = q_pairs + delta[None, None, :]
    kv_pairs = key.reshape(PAIRS, L, D)

    w_args = {
        "wq": np.ascontiguousarray(
            Wq.reshape(KI, 128, H * DK).transpose(1, 0, 2)).astype(NP_DT),
        "wk": np.ascontiguousarray(
            Wk.reshape(KI, 128, H * DK).transpose(1, 0, 2)).astype(NP_DT),
        "wv": np.ascontiguousarray(
            Wv.reshape(KI, 128, H * DK).transpose(1, 0, 2)).astype(NP_DT),
        "wo": np.ascontiguousarray(
            Wo.reshape(KI, 128, D).transpose(1, 0, 2)).astype(NP_DT),
    }

    in_maps = []
    for c in range(N_CORES):
        sl = slice(c * PPC, (c + 1) * PPC)
        m = dict(w_args)
        m["qT"] = _feat_major(q_pairs[sl].reshape(TQ, D))
        m["kvT"] = _feat_major(kv_pairs[sl].reshape(TKV, D))
        in_maps.append(m)

    nc = _build()
    trace = os.environ.get("CCA_TRACE", "") == "1"
    if trace:
        try:
            _install_ntff_hook_shim()
            tmpdir = os.environ.get("CCA_TRACE_DIR") or None
            if tmpdir:
                os.makedirs(tmpdir, exist_ok=True)
            res = run_bass_kernel_spmd(
                nc, in_maps, list(range(N_CORES)), trace=True, tmpdir=tmpdir)
        except Exception as e:
            print(f"trace path failed ({e!r}); rerunning without trace")
            res = run_bass_kernel_spmd(nc, in_maps, list(range(N_CORES)))
    else:
        res = run_bass_kernel_spmd(nc, in_maps, list(range(N_CORES)))
    LAST_RESULTS = res

    # gather: outT [128, OT, TQ] -> [t, D] per core
    out = np.empty((PAIRS, CHUNK, D), dtype=np.float32)
    for c in range(N_CORES):
        oT = res.results[c]["outT"]           # [128, OT, TQ]
        full = oT.transpose(1, 0, 2).reshape(D, TQ)  # [D, t]
        out[c * PPC:(c + 1) * PPC] = full.T.reshape(PPC, CHUNK, D)

    # exact bias correction: bk is a softmax no-op; bv/bo are additive.
    out += (bv.astype(np.float64) @ Wo.astype(np.float64) + bo).astype(
        np.float32)[None, None, :]
    return out.reshape(B, S, D)


# revision 11
# speedup vs baseline: 1.2425x; 1.0024x over previous
"""Chunked cross-attention Trainium2 kernel.

Reference computation (fp32):
  q_in  = query.reshape(B, NC, 64, D)           B=4, NC=32, D=1024
  kv_in = key.reshape(B, NC, 256, D)
  q = (q_in @ Wq + bq)  split into H=16 heads of d_k=64
  k = (kv_in @ Wk + bk), v = (kv_in @ Wv + bv)
  o = softmax(q k^T / 8) v   per (b, chunk, head)
  out = merge_heads(o) @ Wo + bo

Sharding: data-parallel over the 128 independent (b, chunk) pairs,
16 pairs per core, SPMD on 8 cores, no collectives.

Math notes used below:
  * bk is exactly a no-op: scores[q, j] = q_vec . (k_unb[j] + bk) and the
    q_vec . bk term is constant in j, so softmax cancels it.
  * bv and bo are exact post-additive constants: softmax rows sum to 1,
    so o = attn @ (v_unb + bv) = attn @ v_unb + bv, and the final output
    gets + (bv @ Wo + bo), added on the host in fp32.
  * bq folds into the Q-projection PSUM->SBUF copy (per-partition add).

On-chip layouts (per core):
  qT   [128, 8, 1024]  qT[p, ki, t] = q_in^T[ki*128+p, t]   (feature-major)
  kvT  [128, 8, 4096]  same for kv tokens
  w*   [128, 8, 1024]  W[ki*128+p, o]  (natural [in, out])
  Q/K projections emit feature-major [o, t] (o on partitions), which is
  exactly the lhsT/rhs layout the scores matmul needs per head.
  V projection emits token-major [t, o] (tokens on partitions) which is
  the lhsT layout of the attn@V matmul.
  probs are transposed with PE-transpose; O-projection emits out^T
  [D, t] which the host transposes back.
"""

import os

import numpy as np
import ml_dtypes

import concourse.bass as bass
import concourse.mybir as mybir
import concourse.tile as tile
from concourse import bacc
from concourse.bass_utils import run_bass_kernel_spmd
from concourse.masks import make_identity

# ---------------------------------------------------------------- constants
N_CORES = 8
B, S, D = 4, 2048, 1024
NCHUNK, CHUNK, L = 32, 64, 256           # chunks per batch, q len, kv len
H, DK = 16, 64                           # heads, head dim
PAIRS = B * NCHUNK                       # 128 independent (b, chunk) pairs
PPC = PAIRS // N_CORES                   # 16 pairs per core
G = 4                                    # pairs per on-chip group
NG = PPC // G                            # 4 groups
TQ = PPC * CHUNK                         # 1024 q tokens per core
TKV = PPC * L                            # 4096 kv tokens per core
GQ = G * CHUNK                           # 256 q tokens per group
GKV = G * L                              # 1024 kv tokens per group
KI = D // 128                            # 8 contraction tiles
OT = (H * DK) // 128                     # 8 output-feature tiles (2 heads each)

MM_DT = mybir.dt.bfloat16                # matmul operand dtype
NP_DT = ml_dtypes.bfloat16
F32 = mybir.dt.float32

_BUILD_CACHE: dict = {}
LAST_RESULTS = None                      # BassKernelResults of last run (debug)


def _install_ntff_hook_shim():
    """Provide antenv.axon_hooks if the image lacks it (trace-only path).

    Mirrors trn_agent_boot's ctypes NTFF hook against the injected
    libaxon_pjrt.so so run_bass_kernel_spmd(trace=True) can capture
    hardware profiles under axon.
    """
    import sys
    import types
    import ctypes
    import contextlib

    try:
        from antenv.axon_hooks import get_axon_ntff_profile_hook  # noqa: F401
        return
    except ImportError:
        pass

    so_path = "/opt/axon/libaxon_pjrt.so"
    if not os.path.exists(so_path):
        return
    lib = ctypes.CDLL(so_path)
    if not hasattr(lib, "axon_start_nrt_profile"):
        return
    lib.axon_start_nrt_profile.argtypes = [
        ctypes.POINTER(ctypes.c_int64),
        ctypes.c_size_t,
    ]
    lib.axon_start_nrt_profile.restype = ctypes.c_int64
    lib.axon_stop_nrt_profile.argtypes = [ctypes.c_char_p]
    lib.axon_stop_nrt_profile.restype = ctypes.c_int64

    @contextlib.contextmanager
    def _hook(output_dir, device_ids):
        import jax

        jax.devices()
        if device_ids:
            ids = (ctypes.c_int64 * len(device_ids))(*device_ids)
            rc = lib.axon_start_nrt_profile(ids, len(device_ids))
        else:
            rc = lib.axon_start_nrt_profile(None, 0)
        if rc != 0:
            raise RuntimeError(f"axon_start_nrt_profile rc={rc}")
        try:
            yield
        finally:
            n = lib.axon_stop_nrt_profile(str(output_dir).encode())
            print(f"ntff profile: {n} file(s) written to {output_dir}",
                  file=sys.stderr)

    mod = types.ModuleType("antenv.axon_hooks")
    mod.get_axon_ntff_profile_hook = lambda: _hook
    mod.set_axon_ntff_profile_hook = lambda h: None
    import antenv

    antenv.axon_hooks = mod
    sys.modules["antenv.axon_hooks"] = mod


def _emit(nc: bass.Bass, tc, qT_d, kvT_d, wq_d, wk_d, wv_d, wo_d, outT_d):
    import contextlib

    ctx = contextlib.ExitStack()
    with ctx:
        const = ctx.enter_context(tc.tile_pool(name="const", bufs=1))
        w_v = const.tile([128, KI, 1024], MM_DT)
        w_k = const.tile([128, KI, 1024], MM_DT)
        w_q = const.tile([128, KI, 1024], MM_DT)
        w_o = const.tile([128, KI, 1024], MM_DT)
        ident = const.tile([128, 128], MM_DT)
        # w_v first and ki-chunked: V-proj of group 0 is the first PE work
        # and consumes w_v ki-by-ki; chunking lets it start ~6us in.
        for ki in range(KI):
            nc.sync.dma_start(w_v[:, ki, :], wv_d[:, ki, :])
        nc.sync.dma_start(w_k[:], wk_d[:])
        nc.sync.dma_start(w_q[:], wq_d[:])
        nc.sync.dma_start(w_o[:], wo_d[:])
        make_identity(nc, ident[:])

        qio = ctx.enter_context(tc.tile_pool(name="qio", bufs=1))
        qT = qio.tile([128, KI, TQ], MM_DT)
        nc.sync.dma_start(qT[:], qT_d[:])
        qprojT = qio.tile([128, OT, TQ], MM_DT)

        kvin = ctx.enter_context(tc.tile_pool(name="kvin", bufs=2))
        vpp = ctx.enter_context(tc.tile_pool(name="vpp", bufs=2))
        kpp = ctx.enter_context(tc.tile_pool(name="kpp", bufs=3))
        att = ctx.enter_context(tc.tile_pool(name="att", bufs=6))
        otp = ctx.enter_context(tc.tile_pool(name="otp", bufs=2))
        outp = ctx.enter_context(tc.tile_pool(name="outp", bufs=2))
        # PSUM: 8 banks total = psA(2) + ps_s(3) + att2(3)
        psA = ctx.enter_context(tc.tile_pool(name="psA", bufs=2, space="PSUM"))
        psS = ctx.enter_context(tc.tile_pool(name="psS", bufs=3, space="PSUM"))
        psT = ctx.enter_context(tc.tile_pool(name="psT", bufs=3, space="PSUM"))

        def q_projection():
            # qprojT[o, t] : o on partitions (2 heads per ot tile), t free
            for ot in range(OT):
                for c in range(TQ // 512):
                    ps_q = psA.tile([128, 512], F32, tag="psA")
                    for ki in range(KI):
                        nc.tensor.matmul(
                            ps_q[:],
                            w_q[:, ki, ot * 128:(ot + 1) * 128],
                            qT[:, ki, c * 512:(c + 1) * 512],
                            start=(ki == 0),
                            stop=(ki == KI - 1),
                        )
                    nc.vector.tensor_copy(
                        qprojT[:, ot, c * 512:(c + 1) * 512], ps_q[:])

        # ---------------- per-group KV pipeline ---------------------------
        for g in range(NG):
            kvT_g = kvin.tile([128, KI, GKV], MM_DT, tag="kvT_g")
            # token-chunked so the first V-proj groups can start before the
            # whole slab lands (subtile deps track per-chunk readiness)
            for tch in range(4):
                nc.sync.dma_start(
                    kvT_g[:, :, tch * 256:(tch + 1) * 256],
                    kvT_d[:, :, g * GKV + tch * 256: g * GKV + (tch + 1) * 256])

            # V projection, token-major: vproj[t, o], tokens on partitions
            vproj_g = vpp.tile([128, 2 * G, 1024], MM_DT, tag="vproj_g")
            for ts in range(2 * G):
                for oc in range(2):
                    ps_v = psA.tile([128, 512], F32, tag="psA")
                    for ki in range(KI):
                        nc.tensor.matmul(
                            ps_v[:],
                            kvT_g[:, ki, ts * 128:(ts + 1) * 128],
                            w_v[:, ki, oc * 512:(oc + 1) * 512],
                            start=(ki == 0),
                            stop=(ki == KI - 1),
                        )
                    nc.vector.tensor_copy(
                        vproj_g[:, ts, oc * 512:(oc + 1) * 512], ps_v[:])

            if g == 0:
                # behind V-proj g0 so its weight/qT DMAs are covered
                q_projection()

            oT_g = otp.tile([128, OT, GQ], MM_DT, tag="oT_g")

            for ot in range(OT):
                # K projection for this feature tile (2 heads), feature-major
                kprojT = kpp.tile([128, GKV], MM_DT, tag="kprojT")
                for c in range(GKV // 512):
                    ps_k = psA.tile([128, 512], F32, tag="psA")
                    for ki in range(KI):
                        nc.tensor.matmul(
                            ps_k[:],
                            w_k[:, ki, ot * 128:(ot + 1) * 128],
                            kvT_g[:, ki, c * 512:(c + 1) * 512],
                            start=(ki == 0),
                            stop=(ki == KI - 1),
                        )
                    nc.vector.tensor_copy(
                        kprojT[:, c * 512:(c + 1) * 512], ps_k[:])

                # ---- phase 1: scores + softmax for all G pairs -----------
                probs_l = []
                for p in range(G):
                    pg = g * G + p       # global pair index on this core
                    ps_s = psS.tile([128, 256], F32, tag="psS")
                    for h in range(2):
                        sl = slice(h * 64, (h + 1) * 64)
                        nc.tensor.matmul(
                            ps_s[sl, :],
                            qprojT[sl, ot, pg * 64:(pg + 1) * 64],
                            kprojT[sl, p * 256:(p + 1) * 256],
                            start=True, stop=True,
                            tile_position=(h * 64, h * 64),
                        )
                    # softmax along free axis; scores ~ N(0,1), no max-sub
                    probs = att.tile([128, 256], MM_DT, tag="probs")
                    sums = att.tile([128, 1], F32, tag="sums")
                    nc.scalar.activation(
                        probs[:], ps_s[:],
                        mybir.ActivationFunctionType.Exp,
                        bias=0.0, scale=1.0 / np.sqrt(DK).item(),
                        accum_out=sums[:],
                    )
                    rs = att.tile([128, 1], F32, tag="rs")
                    nc.vector.reciprocal(rs[:], sums[:])
                    nc.vector.tensor_scalar_mul(probs[:], probs[:], rs[:])
                    probs_l.append(probs)

                # ---- phase 2: transpose(p) | AV(p-1), lag-1 pipeline ----
                probsT_l = [None] * G
                for p in range(G + 1):
                    if p < G:
                        ps_t = psT.tile([128, 256], MM_DT, tag="psT")
                        nc.tensor.transpose(
                            ps_t[:, 0:128], probs_l[p][:, 0:128], ident[:])
                        nc.tensor.transpose(
                            ps_t[:, 128:256], probs_l[p][:, 128:256], ident[:])
                        probsT = att.tile([128, 256], MM_DT, tag="probsT")
                        nc.vector.tensor_copy(probsT[:], ps_t[:])
                        probsT_l[p] = probsT
                    if p >= 1:
                        pa = p - 1
                        # attn @ V -> o^T [dv2, q], both heads in one MM;
                        # off-diagonal head-cross blocks land unused in PSUM
                        ps_o = psT.tile([128, 128], F32, tag="psT")
                        for kt in range(2):
                            nc.tensor.matmul(
                                ps_o[:],
                                vproj_g[:, pa * 2 + kt,
                                        ot * 128:(ot + 1) * 128],
                                probsT_l[pa][:, kt * 128:(kt + 1) * 128],
                                start=(kt == 0), stop=(kt == 1),
                            )
                        nc.vector.tensor_copy(
                            oT_g[0:64, ot, pa * 64:(pa + 1) * 64],
                            ps_o[0:64, 0:64])
                        nc.vector.tensor_copy(
                            oT_g[64:128, ot, pa * 64:(pa + 1) * 64],
                            ps_o[64:128, 64:128])

            # O projection: out^T [D, t], D on partitions
            outT_g = outp.tile([128, OT, GQ], F32, tag="outT_g")
            for dt in range(OT):
                ps_f = psA.tile([128, 256], F32, tag="psA")
                for ki in range(KI):
                    nc.tensor.matmul(
                        ps_f[:],
                        w_o[:, ki, dt * 128:(dt + 1) * 128],
                        oT_g[:, ki, :],
                        start=(ki == 0),
                        stop=(ki == KI - 1),
                    )
                nc.vector.tensor_copy(outT_g[:, dt, :], ps_f[:])
            nc.sync.dma_start(outT_d[:, :, g * GQ:(g + 1) * GQ], outT_g[:])


def _build():
    key = "k"
    if key in _BUILD_CACHE:
        return _BUILD_CACHE[key]
    nc = bacc.Bacc("TRN2", target_bir_lowering=False, debug=False)
    qT_d = nc.dram_tensor("qT", [128, KI, TQ], MM_DT, kind="ExternalInput").ap()
    kvT_d = nc.dram_tensor("kvT", [128, KI, TKV], MM_DT, kind="ExternalInput").ap()
    wq_d = nc.dram_tensor("wq", [128, KI, 1024], MM_DT, kind="ExternalInput").ap()
    wk_d = nc.dram_tensor("wk", [128, KI, 1024], MM_DT, kind="ExternalInput").ap()
    wv_d = nc.dram_tensor("wv", [128, KI, 1024], MM_DT, kind="ExternalInput").ap()
    wo_d = nc.dram_tensor("wo", [128, KI, 1024], MM_DT, kind="ExternalInput").ap()
    outT_d = nc.dram_tensor("outT", [128, OT, TQ], F32, kind="ExternalOutput").ap()
    with tile.TileContext(nc) as tc:
        _emit(nc, tc, qT_d, kvT_d, wq_d, wk_d, wv_d, wo_d, outT_d)
    nc.compile()
    _BUILD_CACHE[key] = nc
    return nc


def _feat_major(x2d: np.ndarray) -> np.ndarray:
    """[tokens, D] fp32 -> [128, KI, tokens] in MM dtype (d = ki*128 + p)."""
    t, d = x2d.shape
    return np.ascontiguousarray(
        x2d.T.reshape(KI, 128, t).transpose(1, 0, 2)).astype(NP_DT)


def kernel(query, key, Wq, bq, Wk, bk, Wv, bv, Wo, bo):
    global LAST_RESULTS
    query = np.asarray(query, dtype=np.float32)
    key = np.asarray(key, dtype=np.float32)
    Wq = np.asarray(Wq, dtype=np.float32)
    Wk = np.asarray(Wk, dtype=np.float32)
    Wv = np.asarray(Wv, dtype=np.float32)
    Wo = np.asarray(Wo, dtype=np.float32)
    bq = np.asarray(bq, dtype=np.float32)
    bv = np.asarray(bv, dtype=np.float32)
    bo = np.asarray(bo, dtype=np.float32)

    q_pairs = query.reshape(PAIRS, CHUNK, D)
    if np.any(bq):
        # fold bq into the q tokens: (q + delta) @ Wq == q @ Wq + bq
        delta = np.linalg.solve(Wq.astype(np.float64).T,
                                bq.astype(np.float64)).astype(np.float32)
        q_pairs = q_pairs + delta[None, None, :]
    kv_pairs = key.reshape(PAIRS, L, D)

    w_args = {
        "wq": np.ascontiguousarray(
            Wq.reshape(KI, 128, H * DK).transpose(1, 0, 2)).astype(NP_DT),
        "wk": np.ascontiguousarray(
            Wk.reshape(KI, 128, H * DK).transpose(1, 0, 2)).astype(NP_DT),
        "wv": np.ascontiguousarray(
            Wv.reshape(KI, 128, H * DK).transpose(1, 0, 2)).astype(NP_DT),
        "wo": np.ascontiguousarray(
            Wo.reshape(KI, 128, D).transpose(1, 0, 2)).astype(NP_DT),
    }

    in_maps = []
    for c in range(N_CORES):
        sl = slice(c * PPC, (c + 1) * PPC)
        m = dict(w_args)
        m["qT"] = _feat_major(q_pairs[sl].reshape(TQ, D))
        m["kvT"] = _feat_major(kv_pairs[sl].reshape(TKV, D))
        in_maps.append(m)

    nc = _build()
    trace = os.environ.get("CCA_TRACE", "") == "1"
    if trace:
        try:
            _install_ntff_hook_shim()
            tmpdir = os.environ.get("CCA_TRACE_DIR") or None
            if tmpdir:
                os.makedirs(tmpdir, exist_ok=True)
            res = run_bass_kernel_spmd(
                nc, in_maps, list(range(N_CORES)), trace=True, tmpdir=tmpdir)
        except Exception as e:
            print(f"trace path failed ({e!r}); rerunning without trace")
            res = run_bass_kernel_spmd(nc, in_maps, list(range(N_CORES)))
    else:
        res = run_bass_kernel_spmd(nc, in_maps, list(range(N_CORES)))
    LAST_RESULTS = res

    # gather: outT [128, OT, TQ] -> [t, D] per core
    out = np.empty((PAIRS, CHUNK, D), dtype=np.float32)
    for c in range(N_CORES):
        oT = res.results[c]["outT"]           # [128, OT, TQ]
        full = oT.transpose(1, 0, 2).reshape(D, TQ)  # [D, t]
        out[c * PPC:(c + 1) * PPC] = full.T.reshape(PPC, CHUNK, D)

    # exact bias correction: bk is a softmax no-op; bv/bo are additive.
    out += (bv.astype(np.float64) @ Wo.astype(np.float64) + bo).astype(
        np.float32)[None, None, :]
    return out.reshape(B, S, D)
